# revision 7
# baseline (speedup 1.0000x reference)
"""Trainium2 Bass kernel for nn_Exp_loss (exploded-logit / exponomial choice loss).

Math (per assortment row b, S=128 items, derived from the reference; see
kernel_v1 docstring for the step-by-step reduction): in DESCENDING-sorted
space d_0>=d_1>=... with P_i the inclusive prefix sum of d:
    TD_i = P_i - (i+1) d_i
    s    = sum relu(d - chosen)
    cnt  = #{k: d_k >= chosen} = i*+1;  term1 = 1/cnt
    inner = sum_i [d_i < chosen] * exp(min(s - TD_i, 0)) * wd_i,  wd_i = 1/(i(i+1))
    loss_b = log(term1 - inner) - s;  total = -sum_b loss_b / B

Distribution: pure data parallel, 256 rows/core across 8 cores; x is sharded
by item id per row on the host (gather + per-row id sort + one-hot chosen
extraction = pure index/layout work), final loss is a host-side all-reduce.

v3 engine plan:
  - Sort keys are bf16 (host-rounded; numerically validated at 9e-5 rel err
    vs the f32 reference, tolerance is 2e-2).  Halves both the sort-tile DMA
    and the DVE element traffic.
  - Each sort tile's load is split across the two HWDGE queues (SP +
    Activation engines) so the first tile (2x16KB) lands ~1us after issue.
  - DVE: the two 16-round max8/match_replace sorts (the scheduler interleaves
    them), then both tiles' post-chains (scan/tt/stt/ts/recip, reading the bf16 d directly) emitted
    step-interleaved so the chains hide each other's latency.
  - ACT: relu-accumulate (s), exp-accumulate (inner), one final Ln for both
    tiles (exactly two act-table loads, only the Ln one near the tail).
  - Pool: only trivial glue (memset/nch/argc/contrib) -- its ucode tensor ops
    are slow and share the DVE SBUF port.
"""

from contextlib import ExitStack

import numpy as np

import concourse.bass as bass
import concourse.bacc as bacc
import concourse.mybir as mybir
from concourse import tile
from concourse.bass_utils import run_bass_kernel_spmd

B, S = 2048, 128
N = B * S
N_CORES = 8
ROWS_PER_CORE = B // N_CORES          # 256
TILES_PER_CORE = ROWS_PER_CORE // 128  # 2
P = 128
HP = P // 2
NEG_BIG = -1.0e30   # match_replace filler: below any real score
MASK_BIG = 1.0e6    # added (negated) into exp arg to zero masked lanes

F32 = mybir.dt.float32
BF16 = mybir.dt.bfloat16
Alu = mybir.AluOpType
Act = mybir.ActivationFunctionType


def build_program():
    nc = bacc.Bacc()

    gx_d = nc.dram_tensor("gx", [P, TILES_PER_CORE * S], BF16, kind="ExternalInput")
    ch_d = nc.dram_tensor("ch", [P, TILES_PER_CORE], F32, kind="ExternalInput")
    # packed per-core constants: [:, 0:128] ln(wd), [:, 128:256] i+1
    consts_d = nc.dram_tensor("consts", [P, 2 * S], F32, kind="ExternalInput")
    out_d = nc.dram_tensor("partial", [P, TILES_PER_CORE], F32, kind="ExternalOutput")

    with tile.TileContext(nc) as tc, ExitStack() as ctx:
        const = ctx.enter_context(tc.tile_pool(name="const", bufs=1))
        big = ctx.enter_context(tc.tile_pool(name="big", bufs=4))
        work = ctx.enter_context(tc.tile_pool(name="work", bufs=20))
        cols = ctx.enter_context(tc.tile_pool(name="cols", bufs=16))
        fence_deps = []

        # Sort-tile loads, each split across both HWDGE queues by partition
        # half so the first tile lands as early as possible.
        gx_tiles = []
        for t in range(TILES_PER_CORE):
            g = big.tile([P, S], BF16, tag="gx")
            fence_deps.append(nc.sync.dma_start(
                g[0:HP, :], gx_d[0:HP, t * S:(t + 1) * S]))
            fence_deps.append(nc.scalar.dma_start(
                g[HP:P, :], gx_d[HP:P, t * S:(t + 1) * S]))
            gx_tiles.append(g)
        ch_sb = const.tile([P, TILES_PER_CORE], F32)
        fence_deps.append(nc.scalar.dma_start(ch_sb[:], ch_d[:]))
        consts_sb = const.tile([P, 2 * S], F32)
        fence_deps.append(nc.sync.dma_start(consts_sb[:], consts_d[:]))
        lnwd_sb = consts_sb[:, 0:S]
        ip1_sb = consts_sb[:, S:2 * S]

        zeros_sb = const.tile([P, S], F32)
        nc.gpsimd.memset(zeros_sb[:], 0.0)
        # negated chosen scores (bias for the relu(d - chosen) activation)
        nch = const.tile([P, TILES_PER_CORE], F32)
        nc.gpsimd.tensor_scalar(
            out=nch[:], in0=ch_sb[:], scalar1=-1.0, scalar2=None, op0=Alu.mult)

        s2 = const.tile([P, TILES_PER_CORE], F32)
        argc2 = const.tile([P, TILES_PER_CORE], F32)
        act_insts = []

        # ---- DVE: both sorts (scheduler interleaves the two dep chains) ----
        d_tiles = []
        for t in range(TILES_PER_CORE):
            g = gx_tiles[t]
            d = big.tile([P, S], BF16, tag="dsort")
            for k in range(S // 8):
                nc.vector.max(out=d[:, 8 * k:8 * k + 8], in_=g[:])
                if k != S // 8 - 1:
                    nc.vector.match_replace(
                        out=g[:], in_to_replace=d[:, 8 * k:8 * k + 8],
                        in_values=g[:], imm_value=NEG_BIG,
                    )
            d_tiles.append(d)

        # ---- post-chains on DVE/ACT, step-interleaved across tiles ----
        TT = TILES_PER_CORE
        ps = [work.tile([P, S], F32, name=f"ps{t}", tag=f"ps{t}") for t in range(TT)]
        w1 = [work.tile([P, S], F32, name=f"w1{t}", tag=f"w1{t}") for t in range(TT)]
        ntd = [work.tile([P, S], F32, name=f"ntd{t}", tag=f"ntd{t}") for t in range(TT)]
        m = [work.tile([P, S], F32, name=f"m{t}", tag=f"m{t}") for t in range(TT)]
        fge = [work.tile([P, S], F32, name=f"fge{t}", tag=f"fge{t}") for t in range(TT)]
        mm = [work.tile([P, S], F32, name=f"mm{t}", tag=f"mm{t}") for t in range(TT)]
        junk = [work.tile([P, S], F32, name=f"junk{t}", tag=f"junk{t}") for t in range(TT)]
        e = [work.tile([P, S], F32, name=f"e{t}", tag=f"e{t}") for t in range(TT)]
        negs = [cols.tile([P, 1], F32, name=f"negs{t}", tag=f"negs{t}") for t in range(TT)]
        cnt1 = [cols.tile([P, 1], F32, name=f"cnt1{t}", tag=f"cnt1{t}") for t in range(TT)]
        inner = [cols.tile([P, 1], F32, name=f"inner{t}", tag=f"inner{t}") for t in range(TT)]
        t1 = [cols.tile([P, 1], F32, name=f"t1{t}", tag=f"t1{t}") for t in range(TT)]

        for t in range(TT):
            # s = sum relu(d - chosen) via the Relu accumulator (ACT engine)
            act_insts.append(nc.scalar.activation(
                out=junk[t][:], in_=d_tiles[t][:], func=Act.Relu,
                bias=nch[:, t:t + 1], accum_out=s2[:, t:t + 1]))
        for t in range(TT):
            nc.vector.tensor_tensor_scan(
                out=ps[t][:], data0=d_tiles[t][:], data1=zeros_sb[:], initial=0.0,
                op0=Alu.add, op1=Alu.add)
        for t in range(TT):
            nc.vector.tensor_tensor(
                out=w1[t][:], in0=d_tiles[t][:], in1=ip1_sb, op=Alu.mult)
        for t in range(TT):
            nc.vector.tensor_tensor(
                out=ntd[t][:], in0=w1[t][:], in1=ps[t][:], op=Alu.subtract)
        for t in range(TT):
            nc.vector.tensor_scalar(
                out=negs[t][:], in0=s2[:, t:t + 1], scalar1=-1.0, scalar2=None,
                op0=Alu.mult)
        for t in range(TT):
            # m = min(-TD, -s) + ln(wd)
            nc.vector.scalar_tensor_tensor(
                out=m[t][:], in0=ntd[t][:], scalar=negs[t][:], in1=lnwd_sb,
                op0=Alu.min, op1=Alu.add)
        for t in range(TT):
            # fge = [d >= chosen]; row count = i*+1 directly
            nc.vector.tensor_scalar(
                out=fge[t][:], in0=d_tiles[t][:], scalar1=ch_sb[:, t:t + 1],
                scalar2=None, op0=Alu.is_ge, op1=Alu.add, accum_out=cnt1[t][:])
        for t in range(TT):
            # mm = m - MASK_BIG*fge: kept lanes unchanged, others -> -inf-ish
            nc.vector.scalar_tensor_tensor(
                out=mm[t][:], in0=fge[t][:], scalar=-MASK_BIG, in1=m[t][:],
                op0=Alu.mult, op1=Alu.add)
        for t in range(TT):
            # inner = sum exp(mm + s) via the Exp accumulator (ACT engine)
            act_insts.append(nc.scalar.activation(
                out=e[t][:], in_=mm[t][:], func=Act.Exp, bias=s2[:, t:t + 1],
                accum_out=inner[t][:]))
        for t in range(TT):
            nc.vector.reciprocal(out=t1[t][:], in_=cnt1[t][:])
        for t in range(TT):
            nc.gpsimd.tensor_tensor(
                out=argc2[:, t:t + 1], in0=t1[t][:], in1=inner[t][:],
                op=Alu.subtract)

        # single Ln over both tiles' args, contrib = ln - s, one output DMA
        ln2 = const.tile([P, TILES_PER_CORE], F32)
        act_insts.append(
            nc.scalar.activation(out=ln2[:], in_=argc2[:], func=Act.Ln))
        contrib2 = const.tile([P, TILES_PER_CORE], F32)
        nc.gpsimd.tensor_tensor(
            out=contrib2[:], in0=ln2[:], in1=s2[:], op=Alu.subtract)
        fence_deps.append(nc.sync.dma_start(out_d[:], contrib2[:]))

        # Staged SP fences: absorb per-proc completion sems a few at a time so
        # the kernel-tail Drain never carries more sync waits than the CTRL
        # instruction encoding allows.
        fence_deps.extend(act_insts[-2:])
        for i0 in range(0, len(fence_deps), 3):
            nop = nc.sync.nop()
            for dep in fence_deps[i0:i0 + 3]:
                tile.add_dep_helper(nop.ins, dep.ins, sync=True,
                                    reason="tail fence")

    nc.compile()
    return nc


def make_inputs(x, y, assortments):
    """Host-side sharding: per-core input maps (pure index/layout work)."""
    import ml_dtypes
    x = np.ascontiguousarray(np.asarray(x, dtype=np.float32).reshape(N))
    y = np.ascontiguousarray(np.asarray(y, dtype=np.float32).reshape(N))
    a = np.ascontiguousarray(np.asarray(assortments, dtype=np.int32).reshape(B, S))

    i = np.arange(S, dtype=np.float64)
    lnwd = np.full(S, -1.0e4, dtype=np.float32)
    lnwd[1:] = np.log(1.0 / (i[1:] * (i[1:] + 1.0))).astype(np.float32)
    consts = np.ascontiguousarray(np.tile(
        np.concatenate([lnwd, (i + 1.0).astype(np.float32)])[None, :], (P, 1)
    ).astype(np.float32))

    in_maps = []
    for c in range(N_CORES):
        rows = a[c * ROWS_PER_CORE:(c + 1) * ROWS_PER_CORE]  # [256, 128]
        rs = np.sort(rows, axis=1)  # per-row item ids ascending (id-order shard)
        xv16 = x[rs].astype(ml_dtypes.bfloat16)       # [256, S] bf16 sort keys
        cidx = np.argmax(y[rs], axis=1)               # one-hot position per row
        cv = xv16[np.arange(ROWS_PER_CORE), cidx].astype(np.float32)
        gx = np.ascontiguousarray(
            xv16.reshape(TILES_PER_CORE, P, S).transpose(1, 0, 2)
            .reshape(P, TILES_PER_CORE * S))
        ch = np.ascontiguousarray(cv.reshape(TILES_PER_CORE, P).T)
        in_maps.append({"gx": gx, "ch": ch, "consts": consts})
    return in_maps


_PROGRAM_CACHE = {}


def kernel(x, y, assortments, _want_trace=False, _trace_kwargs=None):
    assert np.asarray(x).size == N and np.asarray(assortments).shape == (B, S)
    in_maps = make_inputs(x, y, assortments)
    if "nc" not in _PROGRAM_CACHE:
        _PROGRAM_CACHE["nc"] = build_program()
    nc = _PROGRAM_CACHE["nc"]
    res = run_bass_kernel_spmd(
        nc, in_maps, core_ids=list(range(N_CORES)),
        trace=_want_trace, **(_trace_kwargs or {})
    )
    partials = [np.asarray(res.results[c]["partial"]).reshape(-1).sum(dtype=np.float64) for c in range(N_CORES)]
    total = np.float32(np.sum(np.stack(partials), dtype=np.float64))
    out = np.float32(-total / np.float32(B))
    if _want_trace:
        return out, res
    return out


# revision 8
# speedup vs baseline: 1.1512x; 1.1512x over previous
"""Trainium2 Bass kernel for nn_Exp_loss (exploded-logit / exponomial choice loss).

Math (per assortment row b, S=128 items, derived from the reference; see
kernel_v1 docstring for the step-by-step reduction): in DESCENDING-sorted
space d_0>=d_1>=... with P_i the inclusive prefix sum of d:
    TD_i = P_i - (i+1) d_i
    s    = sum relu(d - chosen)
    cnt  = #{k: d_k >= chosen} = i*+1;  term1 = 1/cnt
    inner = sum_i [d_i < chosen] * exp(min(s - TD_i, 0)) * wd_i,  wd_i = 1/(i(i+1))
    loss_b = log(term1 - inner) - s;  total = -sum_b loss_b / B

Distribution: pure data parallel, 256 rows/core across 8 cores; x is sharded
by item id per row on the host (gather + per-row id sort + one-hot chosen
extraction = pure index/layout work), final loss is a host-side all-reduce.

v3 engine plan:
  - Sort keys are bf16 (host-rounded; numerically validated at 9e-5 rel err
    vs the f32 reference, tolerance is 2e-2).  Halves both the sort-tile DMA
    and the DVE element traffic.
  - Each sort tile's load is split across the two HWDGE queues (SP +
    Activation engines) so the first tile (2x16KB) lands ~1us after issue.
  - DVE: the two 16-round max8/match_replace sorts (the scheduler interleaves
    them), then both tiles' post-chains (copy/scan/tt/stt/ts/recip) emitted
    step-interleaved so the chains hide each other's latency.
  - ACT: relu-accumulate (s), exp-accumulate (inner), one final Ln for both
    tiles (exactly two act-table loads, only the Ln one near the tail).
  - Pool: only trivial glue (memset/nch/argc/contrib) -- its ucode tensor ops
    are slow and share the DVE SBUF port.
"""

from contextlib import ExitStack

import numpy as np

import concourse.bass as bass
import concourse.bacc as bacc
import concourse.mybir as mybir
from concourse import tile
from concourse.bass_utils import run_bass_kernel_spmd

B, S = 2048, 128
N = B * S
N_CORES = 8
ROWS_PER_CORE = B // N_CORES          # 256
TILES_PER_CORE = ROWS_PER_CORE // 128  # 2
P = 128
HP = P // 2
NEG_BIG = -1.0e30   # match_replace filler: below any real score
MASK_BIG = 1.0e6    # added (negated) into exp arg to zero masked lanes

F32 = mybir.dt.float32
BF16 = mybir.dt.bfloat16
Alu = mybir.AluOpType
Act = mybir.ActivationFunctionType


def build_program():
    nc = bacc.Bacc()

    gx_d = nc.dram_tensor("gx", [P, TILES_PER_CORE * S], BF16, kind="ExternalInput")
    ch_d = nc.dram_tensor("ch", [P, TILES_PER_CORE], F32, kind="ExternalInput")
    # packed per-core constants: [:, 0:128] ln(wd), [:, 128:256] i+1
    consts_d = nc.dram_tensor("consts", [P, 2 * S], F32, kind="ExternalInput")
    out_d = nc.dram_tensor("partial", [P, TILES_PER_CORE], F32, kind="ExternalOutput")

    with tile.TileContext(nc) as tc, ExitStack() as ctx:
        const = ctx.enter_context(tc.tile_pool(name="const", bufs=1))
        big = ctx.enter_context(tc.tile_pool(name="big", bufs=4))
        work = ctx.enter_context(tc.tile_pool(name="work", bufs=20))
        cols = ctx.enter_context(tc.tile_pool(name="cols", bufs=16))
        fence_deps = []

        # Sort-tile loads, each split across both HWDGE queues by partition
        # half so the first tile lands as early as possible.
        gx_tiles = []
        for t in range(TILES_PER_CORE):
            g = big.tile([P, S], BF16, tag="gx")
            fence_deps.append(nc.sync.dma_start(
                g[0:HP, :], gx_d[0:HP, t * S:(t + 1) * S]))
            fence_deps.append(nc.scalar.dma_start(
                g[HP:P, :], gx_d[HP:P, t * S:(t + 1) * S]))
            gx_tiles.append(g)
        ch_sb = const.tile([P, TILES_PER_CORE], F32)
        fence_deps.append(nc.scalar.dma_start(ch_sb[:], ch_d[:]))
        consts_sb = const.tile([P, 2 * S], F32)
        fence_deps.append(nc.sync.dma_start(consts_sb[:], consts_d[:]))
        lnwd_sb = consts_sb[:, 0:S]
        ip1_sb = consts_sb[:, S:2 * S]

        zeros_sb = const.tile([P, S], F32)
        nc.gpsimd.memset(zeros_sb[:], 0.0)
        # negated chosen scores (bias for the relu(d - chosen) activation)
        nch = const.tile([P, TILES_PER_CORE], F32)
        nc.gpsimd.tensor_scalar(
            out=nch[:], in0=ch_sb[:], scalar1=-1.0, scalar2=None, op0=Alu.mult)

        s2 = const.tile([P, TILES_PER_CORE], F32)
        argc2 = const.tile([P, TILES_PER_CORE], F32)
        act_insts = []

        # ---- DVE: both sorts (scheduler interleaves the two dep chains) ----
        d_tiles = []
        for t in range(TILES_PER_CORE):
            g = gx_tiles[t]
            d = big.tile([P, S], BF16, tag="dsort")
            for k in range(S // 8):
                nc.vector.max(out=d[:, 8 * k:8 * k + 8], in_=g[:])
                if k != S // 8 - 1:
                    nc.vector.match_replace(
                        out=g[:], in_to_replace=d[:, 8 * k:8 * k + 8],
                        in_values=g[:], imm_value=NEG_BIG,
                    )
            d_tiles.append(d)

        # ---- post-chains on DVE/ACT, step-interleaved across tiles ----
        TT = TILES_PER_CORE
        d32 = [work.tile([P, S], F32, name=f"d32_{t}", tag=f"d32_{t}") for t in range(TT)]
        ps = [work.tile([P, S], F32, name=f"ps{t}", tag=f"ps{t}") for t in range(TT)]
        w1 = [work.tile([P, S], F32, name=f"w1{t}", tag=f"w1{t}") for t in range(TT)]
        ntd = [work.tile([P, S], F32, name=f"ntd{t}", tag=f"ntd{t}") for t in range(TT)]
        m = [work.tile([P, S], F32, name=f"m{t}", tag=f"m{t}") for t in range(TT)]
        fge = [work.tile([P, S], F32, name=f"fge{t}", tag=f"fge{t}") for t in range(TT)]
        mm = [work.tile([P, S], F32, name=f"mm{t}", tag=f"mm{t}") for t in range(TT)]
        junk = [work.tile([P, S], F32, name=f"junk{t}", tag=f"junk{t}") for t in range(TT)]
        e = [work.tile([P, S], F32, name=f"e{t}", tag=f"e{t}") for t in range(TT)]
        negs = [cols.tile([P, 1], F32, name=f"negs{t}", tag=f"negs{t}") for t in range(TT)]
        cnt1 = [cols.tile([P, 1], F32, name=f"cnt1{t}", tag=f"cnt1{t}") for t in range(TT)]
        inner = [cols.tile([P, 1], F32, name=f"inner{t}", tag=f"inner{t}") for t in range(TT)]
        t1 = [cols.tile([P, 1], F32, name=f"t1{t}", tag=f"t1{t}") for t in range(TT)]

        for t in range(TT):
            nc.vector.tensor_copy(out=d32[t][:], in_=d_tiles[t][:])
        for t in range(TT):
            # s = sum relu(d - chosen) via the Relu accumulator (ACT engine)
            act_insts.append(nc.scalar.activation(
                out=junk[t][:], in_=d32[t][:], func=Act.Relu,
                bias=nch[:, t:t + 1], accum_out=s2[:, t:t + 1]))
        for t in range(TT):
            nc.vector.tensor_tensor_scan(
                out=ps[t][:], data0=d32[t][:], data1=zeros_sb[:], initial=0.0,
                op0=Alu.add, op1=Alu.add)
        for t in range(TT):
            nc.vector.tensor_tensor(
                out=w1[t][:], in0=d32[t][:], in1=ip1_sb, op=Alu.mult)
        for t in range(TT):
            nc.vector.tensor_tensor(
                out=ntd[t][:], in0=w1[t][:], in1=ps[t][:], op=Alu.subtract)
        for t in range(TT):
            nc.vector.tensor_scalar(
                out=negs[t][:], in0=s2[:, t:t + 1], scalar1=-1.0, scalar2=None,
                op0=Alu.mult)
        for t in range(TT):
            # m = min(-TD, -s) + ln(wd)
            nc.vector.scalar_tensor_tensor(
                out=m[t][:], in0=ntd[t][:], scalar=negs[t][:], in1=lnwd_sb,
                op0=Alu.min, op1=Alu.add)
        for t in range(TT):
            # fge = [d >= chosen]; row count = i*+1 directly
            nc.vector.tensor_scalar(
                out=fge[t][:], in0=d32[t][:], scalar1=ch_sb[:, t:t + 1],
                scalar2=None, op0=Alu.is_ge, op1=Alu.add, accum_out=cnt1[t][:])
        for t in range(TT):
            # mm = m - MASK_BIG*fge: kept lanes unchanged, others -> -inf-ish
            nc.vector.scalar_tensor_tensor(
                out=mm[t][:], in0=fge[t][:], scalar=-MASK_BIG, in1=m[t][:],
                op0=Alu.mult, op1=Alu.add)
        for t in range(TT):
            # inner = sum exp(mm + s) via the Exp accumulator (ACT engine)
            act_insts.append(nc.scalar.activation(
                out=e[t][:], in_=mm[t][:], func=Act.Exp, bias=s2[:, t:t + 1],
                accum_out=inner[t][:]))
        for t in range(TT):
            nc.vector.reciprocal(out=t1[t][:], in_=cnt1[t][:])
        for t in range(TT):
            nc.gpsimd.tensor_tensor(
                out=argc2[:, t:t + 1], in0=t1[t][:], in1=inner[t][:],
                op=Alu.subtract)

        # single Ln over both tiles' args, contrib = ln - s, one output DMA
        ln2 = const.tile([P, TILES_PER_CORE], F32)
        act_insts.append(
            nc.scalar.activation(out=ln2[:], in_=argc2[:], func=Act.Ln))
        contrib2 = const.tile([P, TILES_PER_CORE], F32)
        nc.gpsimd.tensor_tensor(
            out=contrib2[:], in0=ln2[:], in1=s2[:], op=Alu.subtract)
        fence_deps.append(nc.sync.dma_start(out_d[:], contrib2[:]))

        # Staged SP fences: absorb per-proc completion sems a few at a time so
        # the kernel-tail Drain never carries more sync waits than the CTRL
        # instruction encoding allows.
        fence_deps.extend(act_insts[-2:])
        for i0 in range(0, len(fence_deps), 3):
            nop = nc.sync.nop()
            for dep in fence_deps[i0:i0 + 3]:
                tile.add_dep_helper(nop.ins, dep.ins, sync=True,
                                    reason="tail fence")

    nc.compile()
    return nc


def make_inputs(x, y, assortments):
    """Host-side sharding: per-core input maps (pure index/layout work)."""
    import ml_dtypes
    x = np.ascontiguousarray(np.asarray(x, dtype=np.float32).reshape(N))
    y = np.ascontiguousarray(np.asarray(y, dtype=np.float32).reshape(N))
    a = np.ascontiguousarray(np.asarray(assortments, dtype=np.int32).reshape(B, S))

    i = np.arange(S, dtype=np.float64)
    lnwd = np.full(S, -1.0e4, dtype=np.float32)
    lnwd[1:] = np.log(1.0 / (i[1:] * (i[1:] + 1.0))).astype(np.float32)
    consts = np.ascontiguousarray(np.tile(
        np.concatenate([lnwd, (i + 1.0).astype(np.float32)])[None, :], (P, 1)
    ).astype(np.float32))

    in_maps = []
    for c in range(N_CORES):
        rows = a[c * ROWS_PER_CORE:(c + 1) * ROWS_PER_CORE]  # [256, 128]
        rs = np.sort(rows, axis=1)  # per-row item ids ascending (id-order shard)
        xv16 = x[rs].astype(ml_dtypes.bfloat16)       # [256, S] bf16 sort keys
        cidx = np.argmax(y[rs], axis=1)               # one-hot position per row
        cv = xv16[np.arange(ROWS_PER_CORE), cidx].astype(np.float32)
        gx = np.ascontiguousarray(
            xv16.reshape(TILES_PER_CORE, P, S).transpose(1, 0, 2)
            .reshape(P, TILES_PER_CORE * S))
        ch = np.ascontiguousarray(cv.reshape(TILES_PER_CORE, P).T)
        in_maps.append({"gx": gx, "ch": ch, "consts": consts})
    return in_maps


_PROGRAM_CACHE = {}


def kernel(x, y, assortments, _want_trace=False, _trace_kwargs=None):
    assert np.asarray(x).size == N and np.asarray(assortments).shape == (B, S)
    in_maps = make_inputs(x, y, assortments)
    if "nc" not in _PROGRAM_CACHE:
        _PROGRAM_CACHE["nc"] = build_program()
    nc = _PROGRAM_CACHE["nc"]
    res = run_bass_kernel_spmd(
        nc, in_maps, core_ids=list(range(N_CORES)),
        trace=_want_trace, **(_trace_kwargs or {})
    )
    partials = [np.asarray(res.results[c]["partial"]).reshape(-1).sum(dtype=np.float64) for c in range(N_CORES)]
    total = np.float32(np.sum(np.stack(partials), dtype=np.float64))
    out = np.float32(-total / np.float32(B))
    if _want_trace:
        return out, res
    return out


# revision 9
# speedup vs baseline: 1.1687x; 1.0152x over previous
"""Trainium2 Bass kernel for nn_Exp_loss (exploded-logit / exponomial choice loss).

Math (per assortment row b, S=128 items, derived from the reference; see
kernel_v1 docstring for the step-by-step reduction): in DESCENDING-sorted
space d_0>=d_1>=... with P_i the inclusive prefix sum of d:
    TD_i = P_i - (i+1) d_i
    s    = sum relu(d - chosen)
    cnt  = #{k: d_k >= chosen} = i*+1;  term1 = 1/cnt
    inner = sum_i [d_i < chosen] * exp(min(s - TD_i, 0)) * wd_i,  wd_i = 1/(i(i+1))
    loss_b = log(term1 - inner) - s;  total = -sum_b loss_b / B

Distribution: pure data parallel, 256 rows/core across 8 cores; x is sharded
by item id per row on the host (gather + per-row id sort + one-hot chosen
extraction = pure index/layout work), final loss is a host-side all-reduce.

v3 engine plan:
  - Sort keys are bf16 (host-rounded; numerically validated at 9e-5 rel err
    vs the f32 reference, tolerance is 2e-2).  Halves both the sort-tile DMA
    and the DVE element traffic.
  - Each sort tile's load is split across the two HWDGE queues (SP +
    Activation engines) so the first tile (2x16KB) lands ~1us after issue.
  - DVE: the two 16-round max8/match_replace sorts (the scheduler interleaves
    them), then both tiles' post-chains (copy/scan/tt/stt/ts/recip) emitted
    step-interleaved so the chains hide each other's latency.
  - ACT: relu-accumulate (s), exp-accumulate (inner), one final Ln for both
    tiles (exactly two act-table loads, only the Ln one near the tail).
  - Pool: only trivial glue (memset/nch/argc/contrib) -- its ucode tensor ops
    are slow and share the DVE SBUF port.
"""

from contextlib import ExitStack

import numpy as np

import concourse.bass as bass
import concourse.bacc as bacc
import concourse.mybir as mybir
from concourse import tile
from concourse.bass_utils import run_bass_kernel_spmd

B, S = 2048, 128
N = B * S
N_CORES = 8
ROWS_PER_CORE = B // N_CORES          # 256
TILES_PER_CORE = ROWS_PER_CORE // 128  # 2
P = 128
HP = P // 2
NEG_BIG = -1.0e30   # match_replace filler: below any real score
MASK_BIG = 1.0e6    # added (negated) into exp arg to zero masked lanes

F32 = mybir.dt.float32
BF16 = mybir.dt.bfloat16
Alu = mybir.AluOpType
Act = mybir.ActivationFunctionType


def build_program():
    nc = bacc.Bacc()

    gx_d = nc.dram_tensor("gx", [P, TILES_PER_CORE * S], BF16, kind="ExternalInput")
    ch_d = nc.dram_tensor("ch", [P, TILES_PER_CORE], F32, kind="ExternalInput")
    # packed per-core constants: [:, 0:128] ln(wd), [:, 128:256] i+1
    consts_d = nc.dram_tensor("consts", [P, 2 * S], F32, kind="ExternalInput")
    out_d = nc.dram_tensor("partial", [P, TILES_PER_CORE], F32, kind="ExternalOutput")

    with tile.TileContext(nc) as tc, ExitStack() as ctx:
        const = ctx.enter_context(tc.tile_pool(name="const", bufs=1))
        big = ctx.enter_context(tc.tile_pool(name="big", bufs=4))
        work = ctx.enter_context(tc.tile_pool(name="work", bufs=20))
        cols = ctx.enter_context(tc.tile_pool(name="cols", bufs=16))
        fence_deps = []

        # Sort-tile loads, each split across both HWDGE queues by partition
        # half so the first tile lands as early as possible.
        gx_tiles = []
        for t in range(TILES_PER_CORE):
            g = big.tile([P, S], BF16, tag="gx")
            fence_deps.append(nc.sync.dma_start(
                g[0:HP, :], gx_d[0:HP, t * S:(t + 1) * S]))
            fence_deps.append(nc.scalar.dma_start(
                g[HP:P, :], gx_d[HP:P, t * S:(t + 1) * S]))
            gx_tiles.append(g)
        ch_sb = const.tile([P, TILES_PER_CORE], F32)
        fence_deps.append(nc.scalar.dma_start(ch_sb[:], ch_d[:]))
        consts_sb = const.tile([P, 2 * S], F32)
        fence_deps.append(nc.sync.dma_start(consts_sb[:], consts_d[:]))
        lnwd_sb = consts_sb[:, 0:S]
        ip1_sb = consts_sb[:, S:2 * S]

        zeros_sb = const.tile([P, S], F32)
        nc.gpsimd.memset(zeros_sb[:], 0.0)
        # negated chosen scores (bias for the relu(d - chosen) activation)
        nch = const.tile([P, TILES_PER_CORE], F32)
        nc.gpsimd.tensor_scalar(
            out=nch[:], in0=ch_sb[:], scalar1=-1.0, scalar2=None, op0=Alu.mult)

        s2 = const.tile([P, TILES_PER_CORE], F32)
        argc2 = const.tile([P, TILES_PER_CORE], F32)
        act_insts = []

        # ---- DVE: both sorts (scheduler interleaves the two dep chains) ----
        d_tiles = []
        for t in range(TILES_PER_CORE):
            g = gx_tiles[t]
            d = big.tile([P, S], BF16, tag="dsort")
            for k in range(S // 8):
                nc.vector.max(out=d[:, 8 * k:8 * k + 8], in_=g[:])
                if k != S // 8 - 1:
                    nc.vector.match_replace(
                        out=g[:], in_to_replace=d[:, 8 * k:8 * k + 8],
                        in_values=g[:], imm_value=NEG_BIG,
                    )
            d_tiles.append(d)

        # ---- post-chains on DVE/ACT, step-interleaved across tiles ----
        TT = TILES_PER_CORE
        d32 = [work.tile([P, S], F32, name=f"d32_{t}", tag=f"d32_{t}") for t in range(TT)]
        ps = [work.tile([P, S], F32, name=f"ps{t}", tag=f"ps{t}") for t in range(TT)]
        w1 = [work.tile([P, S], F32, name=f"w1{t}", tag=f"w1{t}") for t in range(TT)]
        ntd = [work.tile([P, S], F32, name=f"ntd{t}", tag=f"ntd{t}") for t in range(TT)]
        m = [work.tile([P, S], F32, name=f"m{t}", tag=f"m{t}") for t in range(TT)]
        fge = [work.tile([P, S], F32, name=f"fge{t}", tag=f"fge{t}") for t in range(TT)]
        mm = [work.tile([P, S], F32, name=f"mm{t}", tag=f"mm{t}") for t in range(TT)]
        junk = [work.tile([P, S], F32, name=f"junk{t}", tag=f"junk{t}") for t in range(TT)]
        e = [work.tile([P, S], F32, name=f"e{t}", tag=f"e{t}") for t in range(TT)]
        negs = [cols.tile([P, 1], F32, name=f"negs{t}", tag=f"negs{t}") for t in range(TT)]
        cnt1 = [cols.tile([P, 1], F32, name=f"cnt1{t}", tag=f"cnt1{t}") for t in range(TT)]
        inner = [cols.tile([P, 1], F32, name=f"inner{t}", tag=f"inner{t}") for t in range(TT)]
        t1 = [cols.tile([P, 1], F32, name=f"t1{t}", tag=f"t1{t}") for t in range(TT)]

        for t in range(TT):
            # bf16->f32 cast on ACT (Copy lives in every act table; keeps the
            # DVE queue free for the scan/tt chain that gates the tail)
            act_insts.append(nc.scalar.activation(
                out=d32[t][:], in_=d_tiles[t][:], func=Act.Copy))
        for t in range(TT):
            # s = sum relu(d - chosen) via the Relu accumulator (ACT engine)
            act_insts.append(nc.scalar.activation(
                out=junk[t][:], in_=d32[t][:], func=Act.Relu,
                bias=nch[:, t:t + 1], accum_out=s2[:, t:t + 1]))
        for t in range(TT):
            nc.vector.tensor_tensor_scan(
                out=ps[t][:], data0=d32[t][:], data1=zeros_sb[:], initial=0.0,
                op0=Alu.add, op1=Alu.add)
        for t in range(TT):
            nc.vector.tensor_tensor(
                out=w1[t][:], in0=d32[t][:], in1=ip1_sb, op=Alu.mult)
        for t in range(TT):
            nc.vector.tensor_tensor(
                out=ntd[t][:], in0=w1[t][:], in1=ps[t][:], op=Alu.subtract)
        for t in range(TT):
            nc.gpsimd.tensor_scalar(
                out=negs[t][:], in0=s2[:, t:t + 1], scalar1=-1.0, scalar2=None,
                op0=Alu.mult)
        for t in range(TT):
            # m = min(-TD, -s) + ln(wd)
            nc.vector.scalar_tensor_tensor(
                out=m[t][:], in0=ntd[t][:], scalar=negs[t][:], in1=lnwd_sb,
                op0=Alu.min, op1=Alu.add)
        for t in range(TT):
            # fge = [d >= chosen]; row count = i*+1 directly
            nc.vector.tensor_scalar(
                out=fge[t][:], in0=d32[t][:], scalar1=ch_sb[:, t:t + 1],
                scalar2=None, op0=Alu.is_ge, op1=Alu.add, accum_out=cnt1[t][:])
        for t in range(TT):
            # mm = m - MASK_BIG*fge: kept lanes unchanged, others -> -inf-ish
            nc.vector.scalar_tensor_tensor(
                out=mm[t][:], in0=fge[t][:], scalar=-MASK_BIG, in1=m[t][:],
                op0=Alu.mult, op1=Alu.add)
        for t in range(TT):
            # inner = sum exp(mm + s) via the Exp accumulator (ACT engine)
            act_insts.append(nc.scalar.activation(
                out=e[t][:], in_=mm[t][:], func=Act.Exp, bias=s2[:, t:t + 1],
                accum_out=inner[t][:]))
        for t in range(TT):
            nc.vector.reciprocal(out=t1[t][:], in_=cnt1[t][:])
        for t in range(TT):
            nc.gpsimd.tensor_tensor(
                out=argc2[:, t:t + 1], in0=t1[t][:], in1=inner[t][:],
                op=Alu.subtract)

        # single Ln over both tiles' args, contrib = ln - s, one output DMA
        ln2 = const.tile([P, TILES_PER_CORE], F32)
        act_insts.append(
            nc.scalar.activation(out=ln2[:], in_=argc2[:], func=Act.Ln))
        contrib2 = const.tile([P, TILES_PER_CORE], F32)
        nc.gpsimd.tensor_tensor(
            out=contrib2[:], in0=ln2[:], in1=s2[:], op=Alu.subtract)
        fence_deps.append(nc.sync.dma_start(out_d[:], contrib2[:]))

        # Staged SP fences: absorb per-proc completion sems a few at a time so
        # the kernel-tail Drain never carries more sync waits than the CTRL
        # instruction encoding allows.
        fence_deps.extend(act_insts[-2:])
        for i0 in range(0, len(fence_deps), 3):
            nop = nc.sync.nop()
            for dep in fence_deps[i0:i0 + 3]:
                tile.add_dep_helper(nop.ins, dep.ins, sync=True,
                                    reason="tail fence")

    nc.compile()
    return nc


def make_inputs(x, y, assortments):
    """Host-side sharding: per-core input maps (pure index/layout work)."""
    import ml_dtypes
    x = np.ascontiguousarray(np.asarray(x, dtype=np.float32).reshape(N))
    y = np.ascontiguousarray(np.asarray(y, dtype=np.float32).reshape(N))
    a = np.ascontiguousarray(np.asarray(assortments, dtype=np.int32).reshape(B, S))

    i = np.arange(S, dtype=np.float64)
    lnwd = np.full(S, -1.0e4, dtype=np.float32)
    lnwd[1:] = np.log(1.0 / (i[1:] * (i[1:] + 1.0))).astype(np.float32)
    consts = np.ascontiguousarray(np.tile(
        np.concatenate([lnwd, (i + 1.0).astype(np.float32)])[None, :], (P, 1)
    ).astype(np.float32))

    in_maps = []
    for c in range(N_CORES):
        rows = a[c * ROWS_PER_CORE:(c + 1) * ROWS_PER_CORE]  # [256, 128]
        rs = np.sort(rows, axis=1)  # per-row item ids ascending (id-order shard)
        xv16 = x[rs].astype(ml_dtypes.bfloat16)       # [256, S] bf16 sort keys
        cidx = np.argmax(y[rs], axis=1)               # one-hot position per row
        cv = xv16[np.arange(ROWS_PER_CORE), cidx].astype(np.float32)
        gx = np.ascontiguousarray(
            xv16.reshape(TILES_PER_CORE, P, S).transpose(1, 0, 2)
            .reshape(P, TILES_PER_CORE * S))
        ch = np.ascontiguousarray(cv.reshape(TILES_PER_CORE, P).T)
        in_maps.append({"gx": gx, "ch": ch, "consts": consts})
    return in_maps


_PROGRAM_CACHE = {}


def kernel(x, y, assortments, _want_trace=False, _trace_kwargs=None):
    assert np.asarray(x).size == N and np.asarray(assortments).shape == (B, S)
    in_maps = make_inputs(x, y, assortments)
    if "nc" not in _PROGRAM_CACHE:
        _PROGRAM_CACHE["nc"] = build_program()
    nc = _PROGRAM_CACHE["nc"]
    res = run_bass_kernel_spmd(
        nc, in_maps, core_ids=list(range(N_CORES)),
        trace=_want_trace, **(_trace_kwargs or {})
    )
    partials = [np.asarray(res.results[c]["partial"]).reshape(-1).sum(dtype=np.float64) for c in range(N_CORES)]
    total = np.float32(np.sum(np.stack(partials), dtype=np.float64))
    out = np.float32(-total / np.float32(B))
    if _want_trace:
        return out, res
    return out
